# revision 19
# baseline (speedup 1.0000x reference)
"""Trainium2 Bass kernel for nn_MetaLEAPPredictor (GNN edge scoring).

reference:
    w0   = sf @ psi_w.T + psi_b                      # [E, 2C]
    coef = w0 + delta_w[li] + u[li]
    s    = sum(x[row] * coef[:, :C], -1) + sum(x[col] * coef[:, C:], -1)
    y    = gamma_h[li][None, :] * leaky_relu(s, 0.01)[:, None]

Algebraic restructure: with b0 = psi_b + delta_w[li] + u[li],
    s_e = <[sf_e, 1], T[row_e, 0:5]> + <[sf_e, 1], T[col_e, 8:13]>
where T = x @ Wext, Wext[c, 0:4] = psi_w[c, :], Wext[c, 4] = b0[c],
Wext[c, 8:12] = psi_w[64+c, :], Wext[c, 12] = b0[64+c]   (c in [0, 64)).

Device plan (8 cores, edges sharded):
  Phase A (each core, full node set): cast x to bf16 (SWDGE cast-DMA),
  DMA-transpose (xbar) packed pairs, PE matmuls vs Wext (bf16) -> PSUM,
  copy to SBUF staging, write table [100096, 64] f32 rows to HBM
  (row index interleaved: trow(n) = ((n%2)*64 + (n%128)//2)*782 + n//128).
  Phase B: per-edge gather of table rows via InstDMAGatherAnt
  (1024 int16 idx per instruction; 16 buckets = row-chunk x col-chunk of
  25024 rows — equal chunks, NOT 32768: a narrow last chunk concentrates
  gather addresses and was measured to both corrupt (flaky engines) and
  serialize on DRAM banks), then DVE mul/reduce/leaky/broadcast.

Reliability (HW-measured): a gather engine's SBUF writes can land AFTER its
completion-sem increment. DVE therefore consumes group G only after group
G+1's gathers (or per-rep dummy gathers after the last group) also
completed. The slack must not cross the nrep boundary: next-rep gathers
transitively wait on vector-engine work queued behind the blocked compute.
Host only shards, buckets, pads, and unpermutes.
"""
import sys
if '/opt/trn_rl_repo' not in sys.path:
    sys.path.insert(0, '/opt/trn_rl_repo')

import numpy as np
import ml_dtypes

import concourse.bacc as bacc
import concourse.bass as bass
import concourse.mybir as mybir
from concourse import tile
from concourse import ap_utils
from concourse.bass import exact_div, round_up_to_multiple
from concourse.library_config import mlp
from concourse.bass_utils import run_bass_kernel_spmd

N = 100000
C = 64
E = 1600000
H = 8
NEG = 0.01
NCORES = 8
NP = 100096            # N padded to 128*782
TILES = NP // 128      # 782
NCHUNKS = 4
CHUNK = NP // NCHUNKS  # 25024 — equal chunks (fits int16; avoids the
                       # narrow-range duplicate-heavy idx lists that showed
                       # flaky gather corruption on the partial last chunk)
GT = 1024              # edges per gather instruction
GS = 8                 # gather tiles per DVE group


import os
GATHER_SP = os.environ.get("KERNEL_SP", "1") == "1"
GATHER_NQ = int(os.environ.get("KERNEL_NQ", "4"))
E32 = os.environ.get("KERNEL_E32", "0") == "1"
GW = 8 if E32 else 16  # gathered f32 lanes per edge


def _dma_gather_raw(gp, out_ap, in_ap, idxs_ap, num_idxs, elem_size, elem_step,
                    queue_num=0):
    """bass.dma_gather minus the (transpose-only) elem%256 assert."""
    assert idxs_ap.dtype == mybir.dt.int16
    assert in_ap.dtype == out_ap.dtype
    assert in_ap.space == bass.MemorySpace.DRAM
    assert ap_utils.ap_is_contiguous(out_ap.ap[1:])
    assert ap_utils.ap_is_contiguous(idxs_ap.ap[1:])
    assert in_ap.ap[-1][1] == out_ap.ap[-1][1] == elem_size
    assert out_ap.ap[0][1] * out_ap.ap[1][1] == round_up_to_multiple(num_idxs, 128)
    assert in_ap.ap[0][0] == elem_step
    stride_bytes_256 = exact_div(elem_step * mybir.dt.size(in_ap.dtype), 256)
    _in_ap = gp.lower_ap_dma(in_ap, for_custom_bir_dma=True)
    return gp.add_instruction(
        mybir.InstDMAGatherAnt(
            name=gp.bass.get_next_instruction_name(),
            ins=[*_in_ap, gp.lower_ap(idxs_ap),
                 gp.lower_val_access(gp.to_reg(num_idxs))],
            outs=[gp.lower_ap(out_ap)],
            transpose=False, num_idxs=num_idxs, elem_size=elem_size,
            stride_bytes_256=stride_bytes_256, gen_mode=0,
            single_packet=GATHER_SP,
            queue_num=queue_num, sbuf_tokens_per_rank=0, sbuf_free_dim_per_rank=0,
            sbuf_free_dim_pad_per_rank=0, sbuf_byte_offset=0,
        ))


def build_program(groups, nrep=1, parts="AGC"):
    """Raw-Bass SPMD program: manual semaphores, 4 SWDGE queues.

    groups: flat list of (bucket, gs); gs = gather tiles (1024 edges each).
    """
    NG = len(groups)
    SL = GT // 128
    IW = GT // 16
    PT = 8
    ngA = (TILES + PT - 1) // PT
    gath_counts = [2 * gs for (_, gs) in groups]
    cum_ga = np.concatenate([[0], np.cumsum(gath_counts)])   # gathers before G
    GA_I = int(cum_ga[-1])
    ga_tot = {}
    ga_dummy = {}
    _q_acc = [0, 0, 0, 0]
    for rep in range(nrep):
        for G in range(NG):
            gs = groups[G][1]
            for t in range(gs):
                _q_acc[(2 * t) % GATHER_NQ] += 16
                _q_acc[(2 * t + 1) % GATHER_NQ] += 16
            ga_tot[(rep, G)] = tuple(_q_acc)
        for q in range(4):
            _q_acc[q] += 16          # per-rep dummy gathers (slack targets)
        ga_dummy[rep] = tuple(_q_acc)
    # slack: consume group G only once G+1's gathers (or the rep's dummies)
    # also landed — guards against an engine's writes landing after its
    # completion-sem increment. Must NOT cross the rep boundary: next-rep
    # gathers transitively wait on vector-engine work queued behind the
    # blocked compute (deadlock).
    ga_wait = {}
    for rep in range(nrep):
        for G in range(NG):
            ga_wait[(rep, G)] = (ga_tot[(rep, G + 1)] if G + 1 < NG
                                 else ga_dummy[rep])
    tw_end = {}
    _tw_acc = [0, 0]
    for rep in range(nrep):
        for g in range(ngA):
            _tw_acc[(rep * ngA + g) % 2] += 16
        tw_end[rep] = tuple(_tw_acc)

    nc = bacc.Bacc("TRN2", target_bir_lowering=False, debug=False,
                   num_devices=NCORES, num_swdge_queues=4,
                   detect_race_conditions=False)

    x32 = nc.dram_tensor("x32", [N, C], mybir.dt.float32, kind="ExternalInput")
    w64 = nc.dram_tensor("w64", [C, 64], mybir.dt.bfloat16, kind="ExternalInput")
    gamma = nc.dram_tensor("gamma", [128, H], mybir.dt.float32,
                           kind="ExternalInput")
    idxr = nc.dram_tensor("idxr", [NG, 128, GS * IW], mybir.dt.int16,
                          kind="ExternalInput")
    idxc = nc.dram_tensor("idxc", [NG, 128, GS * IW], mybir.dt.int16,
                          kind="ExternalInput")
    sfd = nc.dram_tensor("sfd", [NG, 128, GS * SL * 4], mybir.dt.float32,
                         kind="ExternalInput")
    ydev = nc.dram_tensor("ydev", [NG, 128, GS * SL * H], mybir.dt.float32,
                          kind="ExternalOutput")
    xbf = nc.dram_tensor("xbf", [NP * C], mybir.dt.bfloat16)
    # table tail padded to a full chunk multiple: gather src views are always
    # [CHUNK, 16] — partial-chunk views showed flaky engine behavior.
    table = nc.dram_tensor("table", [NCHUNKS * CHUNK, 64], mybir.dt.float32)
    tdump = (nc.dram_tensor("tdump", [NP, 16], mybir.dt.float32,
                            kind="ExternalOutput") if 'D' in parts else None)

    xbf2d = xbf[:].rearrange("(r c) -> r c", c=C)
    xbf_pack = xbf[:].rearrange("(r c) -> r c", c=2 * C)
    table_v = table[0:NP, :].rearrange("(p t) c -> p t c", t=TILES)

    NCAST = 8
    rows_per = (N + NCAST - 1) // NCAST
    CAST_I = NCAST * 16
    TR_I, PE_I, CP_I, TW_I = ngA * 16, ngA, ngA, ngA * 16
    LD_I, CMP_I, ST_I = NG * 48, NG, NG * 16
    PADE = (NP - N) * C // 128

    import contextlib
    with contextlib.ExitStack() as ctx:
        e = ctx.enter_context
        wt = e(nc.sbuf_tensor("wt", [128, 64], mybir.dt.bfloat16))
        gm = e(nc.sbuf_tensor("gm", [128, H], mybir.dt.float32))
        zp = e(nc.sbuf_tensor("zp", [128, PADE], mybir.dt.bfloat16))
        xts = [e(nc.sbuf_tensor(f"xt{i}", [128, PT * 64], mybir.dt.bfloat16))
               for i in range(3)]
        stg = [e(nc.sbuf_tensor(f"stg{i}", [128, PT * 64], mybir.dt.float32))
               for i in range(2)]
        psb = [e(nc.psum_tensor(f"ps{i}", [128, PT * 64], mybir.dt.float32))
               for i in range(4)]
        irs = [e(nc.sbuf_tensor(f"ir{i}", [128, GS * IW], mybir.dt.int16))
               for i in range(3)]
        ics = [e(nc.sbuf_tensor(f"ic{i}", [128, GS * IW], mybir.dt.int16))
               for i in range(3)]
        sfs = [e(nc.sbuf_tensor(f"sf{i}", [128, GS * SL * 4], mybir.dt.float32))
               for i in range(3)]
        grs = [e(nc.sbuf_tensor(f"gr{i}", [128, GS, SL, GW], mybir.dt.float32))
               for i in range(3)]
        gcs = [e(nc.sbuf_tensor(f"gc{i}", [128, GS, SL, GW], mybir.dt.float32))
               for i in range(3)]
        yts = [e(nc.sbuf_tensor(f"yt{i}", [128, GS * SL, H], mybir.dt.float32))
               for i in range(3)]
        scs = [e(nc.sbuf_tensor(f"sc{i}", [128, GS * SL, 6], mybir.dt.float32))
               for i in range(3)]
        gscr = e(nc.sbuf_tensor("gscr", [128, SL, GW], mybir.dt.float32))
        s_ms = e(nc.semaphore("s_ms"))
        s_z = e(nc.semaphore("s_z"))
        s_cast = e(nc.semaphore("s_cast"))
        s_tr = [e(nc.semaphore(f"s_tr{i}")) for i in range(3)]
        s_pe = e(nc.semaphore("s_pe"))
        s_cp = e(nc.semaphore("s_cp"))
        s_tw = [e(nc.semaphore(f"s_tw{i}")) for i in range(2)]
        s_ld = [e(nc.semaphore(f"s_ld{i}")) for i in range(3)]
        s_ga = [e(nc.semaphore(f"s_ga{i}")) for i in range(4)]
        s_cmp = e(nc.semaphore("s_cmp"))
        s_v = e(nc.semaphore("s_v"))
        s_st = [e(nc.semaphore(f"s_st{i}")) for i in range(3)]
        block = e(nc.Block())

        def _twrite(sy, rep, g):
            gg = rep * ngA + g
            t0 = g * PT
            nt = min(PT, TILES - t0)
            sy.wait_ge(s_cp, gg + 1)
            sy.dma_start(
                table_v[:, t0:t0 + nt, :],
                stg[gg % 2][:, :nt * 64].rearrange("p (t c) -> p t c", c=64)
            ).then_inc(s_tw[gg % 2], 16)

        def _loads(sy, gG):
            G = gG % NG
            gs = groups[G][1]
            j = gG % 3
            sy.dma_start(irs[j][:, :gs * IW],
                         idxr[G, :, :gs * IW]).then_inc(s_ld[j], 16)
            sy.dma_start(ics[j][:, :gs * IW],
                         idxc[G, :, :gs * IW]).then_inc(s_ld[j], 16)
            sy.dma_start(sfs[j][:, :gs * SL * 4],
                         sfd[G, :, :gs * SL * 4]).then_inc(s_ld[j], 16)

        @block.sync
        def _(sy):
            sy.dma_start(wt[0:C, :], w64[:]).then_inc(s_ms, 16)
            sy.dma_start(wt[C:2 * C, :], w64[:]).then_inc(s_ms, 16)
            sy.dma_start(gm[:], gamma[:]).then_inc(s_ms, 16)
            sy.wait_ge(s_z, 1)
            sy.dma_start(xbf[N * C:].rearrange("(p c) -> p c", p=128),
                         zp[:]).then_inc(s_ms, 16)
            for rep in range(nrep):
                # phase A
                for g in range(ngA):
                    gg = rep * ngA + g
                    t0 = g * PT
                    nt = min(PT, TILES - t0)
                    prows = nt * 64
                    if gg >= 3:
                        sy.wait_ge(s_pe, gg - 2)
                    if rep == 0 and g == 0:
                        sy.wait_ge(s_ms, 64)
                        sy.wait_ge(s_cast, CAST_I)
                    sy.dma_start_transpose(
                        xts[gg % 3][:, :prows],
                        xbf_pack[t0 * 64: t0 * 64 + prows, :]
                    ).then_inc(s_tr[gg % 3], 16)
                    if g >= 1:
                        _twrite(sy, rep, g - 1)
                _twrite(sy, rep, ngA - 1)
                # phase B stores + lookahead loads (global gG indexing so
                # triple-buffer rotation stays consistent across reps)
                if rep == 0:
                    for gG in range(min(3, nrep * NG)):
                        _loads(sy, gG)
                for G in range(NG):
                    gG = rep * NG + G
                    sy.wait_ge(s_cmp, gG + 1)
                    b, gs = groups[G]
                    nv = gs * SL
                    sy.dma_start(
                        ydev[G, :, :nv * H],
                        yts[gG % 3][:, :nv].rearrange("p s h -> p (s h)")
                    ).then_inc(s_st[gG % 3], 16)
                    if gG + 3 < nrep * NG:
                        _loads(sy, gG + 3)

        @block.gpsimd
        def _(gp):
            gp.load_library(mlp)
            for rep in range(nrep):
                if rep >= 1:
                    gp.wait_ge(s_pe, rep * ngA)
                for i in range(NCAST):
                    r0, r1 = i * rows_per, min((i + 1) * rows_per, N)
                    gp.dma_start(xbf2d[r0:r1, :],
                                 x32[r0:r1, :]).then_inc(s_cast, 16)
                for G in range(NG):
                    gG = rep * NG + G
                    b, gs = groups[G]
                    rc, cc = divmod(b, NCHUNKS)
                    rlo, clo = rc * CHUNK, cc * CHUNK
                    if E32:
                        src_r = table[rlo:rlo + CHUNK, 0:8]
                        src_c = table[clo:clo + CHUNK, 8:16]
                    else:
                        src_r = table[rlo:rlo + CHUNK, 0:16]
                        src_c = table[clo:clo + CHUNK, 0:16]
                    if G == 0:
                        gp.wait_ge(s_tw[0], tw_end[rep][0])
                        gp.wait_ge(s_tw[1], tw_end[rep][1])
                    if gG >= 3:
                        gp.wait_ge(s_cmp, gG - 2)
                    gp.wait_ge(s_ld[gG % 3], (gG // 3 + 1) * 48)
                    for t in range(gs):
                        qr = (2 * t) % GATHER_NQ
                        qc = (2 * t + 1) % GATHER_NQ
                        _dma_gather_raw(
                            gp, grs[gG % 3][:, t], src_r,
                            irs[gG % 3][:, t * IW:(t + 1) * IW], GT, GW, 64,
                            queue_num=qr).then_inc(s_ga[qr], 16)
                        _dma_gather_raw(
                            gp, gcs[gG % 3][:, t], src_c,
                            ics[gG % 3][:, t * IW:(t + 1) * IW], GT, GW, 64,
                            queue_num=qc).then_inc(s_ga[qc], 16)
                    if G == NG - 1:
                        for q in range(4):
                            _dma_gather_raw(
                                gp, gscr[:], src_r,
                                irs[gG % 3][:, 0:IW], GT, GW, 64,
                                queue_num=q).then_inc(s_ga[q], 16)

        @block.tensor
        def _(te):
            te.wait_ge(s_ms, 64)
            for rep in range(nrep):
                for g in range(ngA):
                    gg = rep * ngA + g
                    t0 = g * PT
                    nt = min(PT, TILES - t0)
                    te.wait_ge(s_tr[gg % 3], (gg // 3 + 1) * 16)
                    if gg >= 4:
                        te.wait_ge(s_cp, gg - 3)
                    ps = psb[gg % 4]
                    xt = xts[gg % 3]
                    last = None
                    for j in range(nt):
                        o = j * 64
                        te.matmul(ps[0:64, o:o + 64], xt[0:C, o:o + 64],
                                  wt[0:C, :], start=True, stop=True)
                        last = te.matmul(ps[64:128, o:o + 64],
                                         xt[C:2 * C, o:o + 64],
                                         wt[C:2 * C, :], start=True, stop=True)
                    last.then_inc(s_pe, 1)

        @block.vector
        def _(ve):
            ve.memset(zp[:], 0.0).then_inc(s_z, 1)
            vctr = [0]
            ve.wait_ge(s_ms, 64)
            for rep in range(nrep):
                for g in range(ngA):
                    gg = rep * ngA + g
                    nt = min(PT, TILES - g * PT)
                    ve.wait_ge(s_pe, gg + 1)
                    if gg >= 2:
                        ve.wait_ge(s_tw[gg % 2], (gg - 2) // 2 * 16 + 16)
                    ve.tensor_copy(stg[gg % 2][:, :nt * 64],
                                   psb[gg % 4][:, :nt * 64]).then_inc(s_cp, 1)
                for G in range(NG):
                    gG = rep * NG + G
                    b, gs = groups[G]
                    nv = gs * SL
                    for q in range(4):
                        if ga_wait[(rep, G)][q]:
                            ve.wait_ge(s_ga[q], ga_wait[(rep, G)][q])
                    if gG >= 3:
                        ve.wait_ge(s_st[gG % 3], (gG // 3) * 16)
                    sf4 = sfs[gG % 3][:, :nv * 4].rearrange(
                        "p (s k) -> p s k", k=4)
                    grv = grs[gG % 3][:, :gs].rearrange("p t s e -> p (t s) e")
                    gcv = gcs[gG % 3][:, :gs].rearrange("p t s e -> p (t s) e")
                    yt = yts[gG % 3]
                    sc = scs[gG % 3]
                    pr = sc[:, :nv, 0:4]
                    s0 = sc[:, :nv, 4:5].squeeze(2)
                    s1 = sc[:, :nv, 5:6].squeeze(2)
                    vc = vctr[0]
                    ve.tensor_tensor(out=pr, in0=sf4, in1=grv[:, :, 0:4],
                                     op=mybir.AluOpType.mult).then_inc(s_v, 1)
                    ve.wait_ge(s_v, vc + 1)
                    ve.tensor_reduce(out=s0, in_=pr,
                                     axis=mybir.AxisListType.X,
                                     op=mybir.AluOpType.add).then_inc(s_v, 1)
                    ve.wait_ge(s_v, vc + 2)
                    _cb = 0 if E32 else 8
                    ve.tensor_tensor(out=pr, in0=sf4,
                                     in1=gcv[:, :, _cb:_cb + 4],
                                     op=mybir.AluOpType.mult).then_inc(s_v, 1)
                    ve.wait_ge(s_v, vc + 3)
                    ve.tensor_reduce(out=s1, in_=pr,
                                     axis=mybir.AxisListType.X,
                                     op=mybir.AluOpType.add).then_inc(s_v, 1)
                    ve.wait_ge(s_v, vc + 4)
                    ve.tensor_tensor(out=s0, in0=s0, in1=s1,
                                     op=mybir.AluOpType.add).then_inc(s_v, 1)
                    ve.wait_ge(s_v, vc + 5)
                    ve.tensor_tensor(out=s0, in0=s0,
                                     in1=grv[:, :, 4:5].squeeze(2),
                                     op=mybir.AluOpType.add).then_inc(s_v, 1)
                    ve.wait_ge(s_v, vc + 6)
                    ve.tensor_tensor(out=s0, in0=s0,
                                     in1=gcv[:, :, _cb + 4:_cb + 5].squeeze(2),
                                     op=mybir.AluOpType.add).then_inc(s_v, 1)
                    ve.wait_ge(s_v, vc + 7)
                    ve.scalar_tensor_tensor(out=s0, in0=s0, scalar=NEG,
                                            in1=s0, op0=mybir.AluOpType.mult,
                                            op1=mybir.AluOpType.max
                                            ).then_inc(s_v, 1)
                    ve.wait_ge(s_v, vc + 8)
                    vctr[0] = vc + 8
                    ve.tensor_tensor(
                        out=yt[:, :nv],
                        in0=s0.unsqueeze(2).broadcast_to([128, nv, H]),
                        in1=gm[:].unsqueeze(1).broadcast_to([128, nv, H]),
                        op=mybir.AluOpType.mult).then_inc(s_cmp, 1)

    nc.compile()
    return nc


def _trow(n):
    """table row index for node n (even/odd matmul interleave)."""
    m = n % 128
    return ((m % 2) * 64 + m // 2) * TILES + n // 128


def prep_inputs(x, edge_index, structural_features, layer_idx,
                psi_w, psi_b, delta_w, u, gamma_h):
    li = int(layer_idx)
    b0 = (psi_b + delta_w[li] + u[li]).astype(np.float32)       # [2C]
    w64 = np.zeros((C, 64), dtype=np.float32)
    w64[:, 0:4] = psi_w[:C]
    w64[:, 4] = b0[:C]
    w64[:, 8:12] = psi_w[C:]
    w64[:, 12] = b0[C:]
    w64 = w64.astype(ml_dtypes.bfloat16)
    gamma = np.tile(np.asarray(gamma_h[li], np.float32)[None, :], (128, 1))

    row = np.asarray(edge_index[0], np.int64)
    col = np.asarray(edge_index[1], np.int64)
    sf = np.asarray(structural_features, np.float32)
    x = np.asarray(x, np.float32)

    epc = E // NCORES
    rowt = _trow(row).astype(np.int32)
    colt = _trow(col).astype(np.int32)
    bucket = (rowt // CHUNK) * NCHUNKS + (colt // CHUNK)
    nbuck = NCHUNKS * NCHUNKS
    SL, IW = GT // 128, GT // 16

    import os
    sort_key = os.environ.get("KERNEL_SORT", "none")
    cores = []
    cnts = np.zeros((NCORES, nbuck), dtype=np.int64)
    for c in range(NCORES):
        sl = slice(c * epc, (c + 1) * epc)
        if sort_key == "row":
            key = bucket[sl].astype(np.int64) * (1 << 20) + rowt[sl]
        elif sort_key == "col":
            key = bucket[sl].astype(np.int64) * (1 << 20) + colt[sl]
        else:
            key = bucket[sl]
        order = np.argsort(key, kind='stable') + c * epc
        cnts[c] = np.bincount(bucket[sl], minlength=nbuck)
        cores.append(order)
    tpb_b = np.maximum(1, -(-cnts.max(axis=0) // GT))           # [nbuck]
    groups = []
    for b in range(nbuck):
        t = int(tpb_b[b])
        while t > 0:
            gs = min(GS, t)
            groups.append((b, gs))
            t -= gs
    ngroups = len(groups)

    in_maps = []
    eid_all = []
    for c in range(NCORES):
        order = cores[c]
        idxr_d = np.zeros((ngroups, 128, GS * IW), dtype=np.int16)
        idxc_d = np.zeros((ngroups, 128, GS * IW), dtype=np.int16)
        sfd_d = np.zeros((ngroups, 128, GS * SL * 4), dtype=np.float32)
        eids_d = np.full((ngroups, GS * GT), -1, dtype=np.int64)
        boff = np.concatenate([[0], np.cumsum(cnts[c])])
        tile_done = {b: 0 for b in range(nbuck)}
        for g, (b, gs) in enumerate(groups):
            t0 = tile_done[b]
            tile_done[b] = t0 + gs
            lo = boff[b] + t0 * GT
            hi = min(boff[b + 1], lo + gs * GT)
            cnt = max(0, int(hi - lo))
            ids = order[lo:hi]
            npad = gs * GT
            rl = np.zeros(npad, dtype=np.int16)
            cl = np.zeros(npad, dtype=np.int16)
            rl[:cnt] = (rowt[ids] % CHUNK).astype(np.int16)
            cl[:cnt] = (colt[ids] % CHUNK).astype(np.int16)
            eids_d[g, :cnt] = ids
            sfp = np.zeros((npad, 4), dtype=np.float32)
            sfp[:cnt] = sf[ids]
            # idx wrap: [gs, GT] -> per tile [16, IW] replicated to 128
            def wrap(a):
                a = a.reshape(gs, IW, 16).transpose(0, 2, 1)    # [gs, 16, IW]
                a = np.tile(a, (1, 8, 1))                       # [gs, 128, IW]
                return a.transpose(1, 0, 2).reshape(128, gs * IW)
            idxr_d[g, :, :gs * IW] = wrap(rl)
            idxc_d[g, :, :gs * IW] = wrap(cl)
            # sf: edge m = s*128+p of tile t
            sfd_d[g, :, :gs * SL * 4] = (
                sfp.reshape(gs, SL, 128, 4).transpose(2, 0, 1, 3)
                .reshape(128, gs * SL * 4))
        in_maps.append({
            "x32": x, "w64": w64, "gamma": gamma,
            "idxr": idxr_d, "idxc": idxc_d, "sfd": sfd_d,
        })
        eid_all.append(eids_d)
    return in_maps, eid_all, groups


def unshard(results, eid_all, groups):
    SL = GT // 128
    y = np.empty((E, H), dtype=np.float32)
    for c in range(NCORES):
        yd = results[c]["ydev"]          # [ngroups, 128, GS*SL*H]
        eids = eid_all[c]                # [ngroups, GS*GT]
        for g, (b, gs) in enumerate(groups):
            blk = yd[g, :, :gs * SL * H].reshape(128, gs, SL, H)
            blk = blk.transpose(1, 2, 0, 3).reshape(gs * GT, H)
            ids = eids[g, :gs * GT]
            valid = ids >= 0
            y[ids[valid]] = blk[valid]
    return y


_CACHE = {}


def kernel(**inputs):
    in_maps, eid_all, groups = prep_inputs(**inputs)
    key = tuple(groups)
    if key not in _CACHE:
        _CACHE[key] = build_program(groups)
    nc = _CACHE[key]
    res = run_bass_kernel_spmd(nc, in_maps, core_ids=list(range(NCORES)))
    return unshard(res.results, eid_all, groups)

